# revision 59
# baseline (speedup 1.0000x reference)
"""Trainium2 Bass kernel for nn_NodeAttDiff (segment-reduce node attention).

Math (reference):
    e1, e2 = out_gnn[:N], out_gnn[N:]          # N = 200000, D = 256
    diff   = e1 - e2
    h      = relu([e1 e2 diff] @ W1 + b1)      # folded: e1@WA + e2@WB, WA=W1a+W1c, WB=W1b-W1c
    raw    = (h @ W2 + b2)[:, 0]
    att    = segment_softmax(raw, batch)       # 512 contiguous segments (batch sorted)
    out    = segment_sum(att[:,None] * diff)   # [512, 256]

Device strategy (8 cores, graph-partitioned data parallel):
    - 64 graphs / core; each core gets its contiguous node slice (padded to a
      common capacity, pad nodes carry out-of-range segment id -> dropped).
    - Softmax max-subtraction is skipped (raw is O(5); exp is safe in fp32) and
      normalization is algebraic:  out_g = (sum_n w_n diff_n) / (sum_n w_n),
      w_n = exp(raw_n + b2).
    - W2 is folded into the z GEMM on the host:  columns of WA/WB are permuted
      so features with w2 >= 0 come first (P of them) and scaled by |w2|, so
        raw = sum_{f<P} relu(z'_f) - sum_{f>=P} relu(z'_f)
      i.e. raw is a pair of free-axis reductions -- no [2,512] raw matmul, and
      because z' is computed NODE-major (e-chunks stationary, W' moving; same
      total PE cycles) raw lands node-major, so the exp weights need no
      transpose at all (the old I4 outer-product matmuls are gone too).
    - Host ships e1/e2 feature-major fp16 (merged e12, stationary source) and
      dn node-major [cap/128,128,258] = [diff | 1 | 0] rows that feed the
      segment matmul directly; ~39 MB total vs 360 GB/s HBM.
    - Software pipeline, one group = 2x512-node tiles; every cross-engine
      dependency gets >= 1 full iteration of slack so the in-order PE queue
      never stalls and the tensor engine stays at its ramped p-state:
        iteration g:  PE:  seg(g-3) x8 -> z(g) x32 (node-major, 256-col mms)
                      ACT: relu(g) x2, exp(g-2) x2 ([128,4] each)
                      DVE: reduce/sub(g-2) x6, Sw(g-2) x8
      PSUM: 3x z tiles (2 banks each) + seg accumulator = 7 banks.
    - Tail: out = seg[:,0:256] * recip(max(seg[:,256], eps)), DMA out [gw,256].
"""

import os
import ml_dtypes
import numpy as np

NUM_GRAPHS = 512
N_CORES = 8
D = 256
TILE_N = 512  # nodes per tile
DN_W = D + 2  # diff row + [1, 0]


_CACHE = {}


def _build_program(cap: int, gw: int, P: int, use_b1: bool = False):
    """Build + compile the SPMD Bass program; `cap` nodes and a `gw`-graph
    window per core; `P` = number of w2>=0 output features (sign split)."""
    key = (cap, gw, P, use_b1)
    if key in _CACHE:
        return _CACHE[key]

    from contextlib import ExitStack
    import concourse.bass as bass
    import concourse.tile as tile
    import concourse.bacc as bacc
    import concourse.mybir as mybir

    f32 = mybir.dt.float32
    f16 = mybir.dt.float16
    f8 = mybir.dt.float8e3  # e3m4: 4 mantissa bits, halves e12 HBM traffic
    AF = mybir.ActivationFunctionType
    ALU = mybir.AluOpType
    AX = mybir.AxisListType

    assert cap % (2 * TILE_N) == 0
    n_grp = cap // (2 * TILE_N)
    n_cols = cap // 128  # bm columns
    n_blk = cap // 128   # 128-node blocks

    nc = bacc.Bacc("TRN2", target_bir_lowering=False, debug=False,
                   num_devices=N_CORES)

    e12_d = nc.dram_tensor("e12", [2, 2, 128, cap], f8, kind="ExternalInput").ap()
    dn_d = nc.dram_tensor("dn", [n_blk, 128, DN_W], f16, kind="ExternalInput").ap()
    bm_d = nc.dram_tensor("bm", [128, n_cols], f32, kind="ExternalInput").ap()
    wmov_d = nc.dram_tensor("wmov", [2, 2, 128, D], f16, kind="ExternalInput").ap()
    b1bc_d = nc.dram_tensor("b1bc", [128, D], f32, kind="ExternalInput").ap()
    b2c_d = nc.dram_tensor("b2c", [128, 1], f32, kind="ExternalInput").ap()
    iota_d = nc.dram_tensor("iota", [128, gw], f16, kind="ExternalInput").ap()
    out_d = nc.dram_tensor("out", [gw, D], f32, kind="ExternalOutput").ap()

    with tile.TileContext(nc) as tc:
        with ExitStack() as ctx:
            consts = ctx.enter_context(tc.tile_pool(name="consts", bufs=1))
            epool = ctx.enter_context(tc.tile_pool(name="epool", bufs=8))
            dpool = ctx.enter_context(tc.tile_pool(name="dpool", bufs=8))
            hpool = ctx.enter_context(tc.tile_pool(name="hpool", bufs=6))
            spool = ctx.enter_context(tc.tile_pool(name="spool", bufs=6))
            zpool = ctx.enter_context(
                tc.tile_pool(name="zpool", bufs=3, space=bass.MemorySpace.PSUM))
            segpool = ctx.enter_context(
                tc.tile_pool(name="segpool", bufs=1, space=bass.MemorySpace.PSUM))

            # ---- constants ----
            wmov = consts.tile([128, 2, 2, D], f16, tag="wmov")
            b1bc = consts.tile([128, D], f32, tag="b1bc")
            b2c = consts.tile([128, 1], f32, tag="b2c")
            iota = consts.tile([128, gw], f16, tag="iota")
            bm = consts.tile([128, n_cols], f32, tag="bm")

            # seg layout: cols 0:256 weighted diff sums, col 256 exp-sums
            seg = segpool.tile([gw, DN_W], f32, tag="seg")

            e_tiles = {}
            d_tiles = {}

            def issue_edma(g):
                if g >= n_grp:
                    return
                e12 = epool.tile([128, 2, 2, 2 * TILE_N], f8, tag="e12")
                gsl = bass.ts(g, 2 * TILE_N)
                nc.sync.dma_start(
                    e12[:], e12_d[:, :, :, gsl].rearrange("s k p n -> p s k n"))
                e_tiles[g] = e12

            def issue_ddma(g):
                if g >= n_grp:
                    return
                dn = dpool.tile([128, 8, DN_W], f16, tag="dn")
                nc.sync.dma_start(dn[:], dn_d[bass.ts(g, 8)].rearrange("b p f -> p b f"))
                d_tiles[g] = dn

            def issue_dma(g):
                issue_edma(g)
                issue_ddma(g)

            # startup criticial path: z(0) needs only e12(0) + wmov; dn is
            # first consumed at pipeline iteration 3, so it dispatches after
            issue_edma(0)
            nc.sync.dma_start(wmov[:], wmov_d.rearrange("k s p n -> p k s n"))
            issue_edma(1)
            issue_ddma(0)
            issue_ddma(1)
            issue_dma(2)
            # remaining consts (none are needed before pipeline iteration 2,
            # and behind e12(2) they stay off the startup bandwidth burst)
            nc.sync.dma_start(b1bc[:], b1bc_d[:])
            nc.sync.dma_start(b2c[:], b2c_d[:])
            nc.sync.dma_start(iota[:], iota_d[:])
            nc.sync.dma_start(bm[:], bm_d[:])

            h_tiles = {}
            sw_tiles = {}
            seg_started = [False]

            # every cross-engine dependency gets >= 1 full iteration of slack:
            # the reduce chain consumes h(g) two iterations after relu(g) was
            # issued, seg(g) consumes sw(g) one full iteration after the DVE
            # batch that produced it, so the in-order PE queue never stalls.
            # Drain is compressed by one iteration: the last group's reduce
            # chain runs at it == n_grp (all engines idle there, and its relu
            # completes at the top of that iteration) and both trailing seg
            # groups run at it == n_grp + 1.
            for it in range(n_grp + 2):
                gz = it          # z / relu phase
                if it == n_grp - 1 and n_grp >= 2:
                    grs = [g for g in (it - 2, n_grp - 2) if g >= 0]
                elif it == n_grp:
                    grs = [n_grp - 1]
                elif 2 <= it < n_grp - 1:
                    grs = [it - 2]
                else:
                    grs = []
                if it == n_grp:
                    gss = [g for g in (it - 3, n_grp - 2) if g >= 0]
                elif it == n_grp + 1:
                    gss = [n_grp - 1]
                elif 3 <= it < n_grp:
                    gss = [it - 3]
                else:
                    gss = []
                gss = sorted(set(gss))
                grs = sorted(set(grs))

                # ---- prefetch group g+3 (3 groups of DMA runahead)
                issue_dma(it + 3)

                # ---- raw(g-2) = sum_pos relu(z') - sum_neg relu(z'):
                # free-axis reductions on DVE, exp on ACT ([128,4] node-major)
                for gr in grs:
                    ewts = []
                    for ti in range(2):
                        h = h_tiles.pop((gr, ti))
                        raw = spool.tile([128, 4], f32, tag="raw")
                        if P == 0:
                            rB = spool.tile([128, 4], f32, tag="rB")
                            nc.vector.tensor_reduce(rB[:], h[:], axis=AX.X,
                                                    op=ALU.add)
                            nc.vector.tensor_scalar_mul(raw[:], rB[:], -1.0)
                        elif P == D:
                            nc.vector.tensor_reduce(raw[:], h[:], axis=AX.X,
                                                    op=ALU.add)
                        else:
                            rA = spool.tile([128, 4], f32, tag="rA")
                            rB = spool.tile([128, 4], f32, tag="rB")
                            nc.vector.tensor_reduce(rA[:], h[:, :, 0:P],
                                                    axis=AX.X, op=ALU.add)
                            nc.vector.tensor_reduce(rB[:], h[:, :, P:D],
                                                    axis=AX.X, op=ALU.add)
                            nc.vector.tensor_sub(raw[:], rA[:], rB[:])
                        # ewt = exp(raw + b2), already node-major [128, 4]
                        ewt = spool.tile([128, 4], f32, tag="ewt")
                        nc.scalar.activation(ewt[:], raw[:], AF.Exp,
                                             bias=b2c[:], scale=1.0)
                        ewts.append(ewt)
                    # Sw[:, b, :] = (iota == bm_col) * ewt_col
                    for ti in range(2):
                        t = 2 * gr + ti
                        sw = spool.tile([128, 4, gw], f16, tag="sw")
                        for b in range(4):
                            nc.vector.tensor_scalar(
                                sw[:, b, :], iota[:],
                                bm[:, 4 * t + b:4 * t + b + 1],
                                ewts[ti][:, b:b + 1],
                                op0=ALU.is_equal, op1=ALU.mult)
                        sw_tiles[(gr, ti)] = sw

                # ---- seg(g-3): 8 matmuls into the whole-core accumulator
                for gs in gss:
                    dnt = d_tiles.pop(gs)
                    for ti in range(2):
                        sw = sw_tiles.pop((gs, ti))
                        for b in range(4):
                            nc.tensor.matmul(seg[:], sw[:, b, :],
                                             dnt[:, 4 * ti + b, :],
                                             start=not seg_started[0],
                                             stop=(gs == n_grp - 1 and ti == 1
                                                   and b == 3),
                                             skip_group_check=True)
                            seg_started[0] = True

                # ---- z(g): node-major, 16 matmuls/tile of 256 cols
                # (e-chunk [128f,128n] stationary, W' [128f,256] moving)
                if gz < n_grp:
                    e12 = e_tiles.pop(gz)
                    for ti in range(2):
                        zc = zpool.tile([128, 4, D], f32, tag="zr",
                                        name=f"z_{gz}_{ti}")
                        for c in range(4):
                            nsl = bass.ts(4 * ti + c, 128)
                            for j, (s, k) in enumerate(
                                    [(0, 0), (0, 1), (1, 0), (1, 1)]):
                                nc.tensor.matmul(
                                    zc[:, c, :], e12[:, s, k, nsl],
                                    wmov[:, k, s, :],
                                    start=(j == 0), stop=(j == 3))
                        if use_b1:
                            # generic path: z' += |w2|*b1 broadcast over nodes
                            for c in range(4):
                                nc.vector.tensor_add(zc[:, c, :], zc[:, c, :],
                                                     b1bc[:])
                        # h' = relu(z'), single ACT over both PSUM banks
                        h = hpool.tile([128, 4, D], f16, tag="h")
                        nc.scalar.activation(h[:], zc[:], AF.Relu, scale=1.0)
                        h_tiles[(gz, ti)] = h

            # tail: out = seg[:, 0:256] / max(seg[:, 256], eps)
            ssum = spool.tile([gw, 1], f32, tag="ssum")
            nc.vector.tensor_scalar_max(ssum[:], seg[:, D:D + 1], 1e-30)
            rec = spool.tile([gw, 1], f32, tag="rec")
            nc.vector.reciprocal(rec[:], ssum[:])
            ot = spool.tile([gw, D], f32, tag="ot")
            nc.vector.tensor_scalar_mul(ot[:], seg[:, 0:D], rec[:])
            nc.sync.dma_start(out_d[:], ot[:])

    nc.compile()
    _CACHE[key] = nc
    return nc


def _prepare(out_gnn, batch_input, W1, b1, W2, b2):
    out_gnn = np.asarray(out_gnn, dtype=np.float32)
    batch = np.asarray(batch_input, dtype=np.int64)
    W1 = np.asarray(W1, dtype=np.float32)
    b1 = np.asarray(b1, dtype=np.float32)
    W2 = np.asarray(W2, dtype=np.float32)
    b2 = np.asarray(b2, dtype=np.float32)
    use_b1 = bool(b1.any())

    half = out_gnn.shape[0] // 2
    batch = batch[:half]
    e1_all, e2_all = out_gnn[:half], out_gnn[half:]

    # Node-balanced, graph-aligned contiguous cuts. Core c handles graphs
    # [gcut[c], gcut[c+1]) and the matching contiguous node range. The
    # sorted batch may populate only a prefix of the 512 graphs, so cuts
    # are chosen by node mass, not by fixed graph ranges.
    counts = np.bincount(batch, minlength=NUM_GRAPHS)
    ccum = np.concatenate([[0], np.cumsum(counts)])  # node offset per graph
    # only graphs up to the last populated one get device windows; trailing
    # empty graphs stay host-side zeros
    g_used = int(np.max(np.nonzero(counts)[0])) + 1 if counts.any() else 1
    gcut = np.zeros(N_CORES + 1, dtype=np.int64)
    gcut[N_CORES] = g_used
    for c in range(1, N_CORES):
        g = int(np.searchsorted(ccum, ccum[g_used] * c / N_CORES, side="left"))
        gcut[c] = min(max(g, gcut[c - 1]), g_used)
    spans = gcut[1:] - gcut[:-1]
    if spans.max() > 128:
        # node-balanced cuts gave an oversized graph window (pathological
        # distribution) -- fall back to an even graph split of [0, g_used)
        gcut = np.round(np.linspace(0, g_used, N_CORES + 1)).astype(np.int64)
        spans = gcut[1:] - gcut[:-1]
        if spans.max() > 128:
            raise ValueError(f"graph window {spans.max()} > 128 unsupported")

    nbounds = ccum[gcut]  # node boundaries per core
    gw = int(max(2, ((spans.max() + 1) // 2) * 2))
    max_n = int((nbounds[1:] - nbounds[:-1]).max())
    grp = 2 * TILE_N
    cap = max(grp, ((max_n + grp - 1) // grp) * grp)

    # fold W2 into the z GEMM: permute output features (w2 >= 0 first) and
    # scale columns by |w2|, so raw = sum(pos cols) - sum(neg cols) of relu(z')
    W1a = W1[0:D].astype(np.float64)
    W1b = W1[D:2 * D].astype(np.float64)
    W1c = W1[2 * D:3 * D].astype(np.float64)
    WA = W1a + W1c
    WB = W1b - W1c
    w2 = W2[:, 0].astype(np.float64)
    perm = np.argsort(w2 < 0, kind="stable")  # stable: w2 >= 0 first
    P = int((w2 >= 0).sum())
    WAp = (WA[:, perm] * np.abs(w2[perm])).astype(np.float16)
    WBp = (WB[:, perm] * np.abs(w2[perm])).astype(np.float16)

    nc = _build_program(cap, gw, P, use_b1)

    wmov = np.zeros((2, 2, 128, D), dtype=np.float16)  # [k, s, 128, D]
    for k in range(2):
        wmov[k, 0] = WAp[128 * k:128 * (k + 1), :]
        wmov[k, 1] = WBp[128 * k:128 * (k + 1), :]

    b1p = (np.abs(w2[perm]) * b1.astype(np.float64)[perm]).astype(np.float32)
    common = {
        "wmov": np.ascontiguousarray(wmov),
        "b1bc": np.broadcast_to(b1p, (128, D)).copy(),
        "b2c": np.full((128, 1), b2[0], dtype=np.float32),
        "iota": np.broadcast_to(np.arange(gw, dtype=np.float16), (128, gw)).copy(),
    }

    in_maps = []
    for c in range(N_CORES):
        s, e = int(nbounds[c]), int(nbounds[c + 1])
        n_c = e - s
        f8np = ml_dtypes.float8_e3m4
        e12 = np.zeros((2, 2, 128, cap), dtype=f8np)
        e12[0, :, :, :n_c] = e1_all[s:e].astype(f8np).T.reshape(2, 128, n_c)
        e12[1, :, :, :n_c] = e2_all[s:e].astype(f8np).T.reshape(2, 128, n_c)
        dn = np.zeros((cap, DN_W), dtype=np.float16)
        dn[:n_c, :D] = (e1_all[s:e] - e2_all[s:e]).astype(np.float16)
        dn[:, D] = 1.0  # denominator ones column (pad rows get Sw == 0 anyway)
        bmv = np.full(cap, 999.0, dtype=np.float32)
        bmv[:n_c] = (batch[s:e] - gcut[c]).astype(np.float32)
        in_maps.append({
            "e12": e12,
            "dn": dn.reshape(cap // 128, 128, DN_W),
            "bm": np.ascontiguousarray(bmv.reshape(cap // 128, 128).T),
            **common,
        })
    return nc, in_maps, gcut


def _enable_ldw_opt():
    """Re-enable the compiler's weight-load optimization (off by default in
    this container's flag set); harmless no-op if the flag isn't present."""
    try:
        from concourse.compiler_utils import get_compiler_flags, set_compiler_flags
        flags = [f.replace("--enable-ldw-opt=false", "--enable-ldw-opt=true")
                 for f in get_compiler_flags()]
        set_compiler_flags(flags)
    except Exception:
        pass


def kernel(out_gnn, batch_input, W1, b1, W2, b2):
    import concourse.bass_utils as bass_utils

    _enable_ldw_opt()
    nc, in_maps, gcut = _prepare(out_gnn, batch_input, W1, b1, W2, b2)

    trace_dir = os.environ.get("NODEATT_TRACE_DIR")
    kw = {}
    if trace_dir:
        kw = {"trace": True, "tmpdir": trace_dir}
    res = bass_utils.run_bass_kernel_spmd(
        nc, in_maps, core_ids=list(range(N_CORES)), **kw)
    if trace_dir:
        kernel.last_exec_time_ns = res.exec_time_ns
        kernel.last_results = res

    out = np.zeros((NUM_GRAPHS, D), dtype=np.float32)
    for c in range(N_CORES):
        span = int(gcut[c + 1] - gcut[c])
        if span > 0:
            out[gcut[c]:gcut[c + 1]] = res.results[c]["out"][:span]
    return out
